# revision 29
# baseline (speedup 1.0000x reference)
"""Trainium2 Bass kernel for nn_CrossAttention (B=2, N=1024, L=4096, C=1024, H=16).

Single fused program, head-parallel across 8 NeuronCores (2 heads per core),
bf16 matmuls with f32 PSUM accumulation:
  - q/k projections as [dim, row] matmuls; v projected directly into natural
    [row, dim] layout (no PE transposes).
  - full NxL attention for the core's 2 heads; softmax denominator via an
    appended ones-column in V; exp on the scalar engine (bf16 out), with the
    AV matmuls software-pipelined one step behind the score matmuls so the
    scalar engine never starves the PE.
  - batch-1 k/v projection units are interleaved into both batches' attention
    loops (with prefetched y loads) to fill ACT-bound PE gaps.
  - normalized attention outputs for batch 0 (full width) and batch 1 cols
    0:512 (half width) are exchanged with in-kernel AllToAlls that overlap
    later compute; each core projects its row slice with the full Wp.
  - the final block (batch 1 cols 512:1024) skips the tail-exposed collective:
    each core emits its partial projection over its 128 local dims and the
    host sums the 8 partials (+bias) for those 512 rows.
"""

import functools

import numpy as np

B, N, L, C = 2, 1024, 4096, 1024
H, D = 16, 64
SCALE = D ** -0.5
NCORES = 8
LOCD = C // NCORES       # 128 local head-dims per core (2 heads x 64)
R = B * N                # 2048 query rows
RL = B * L               # 8192 key rows
KT = C // 128            # 8 contraction tiles


def _split_excess_waits(nc, max_waits=1):
    """walrus in this container rejects >1 sync wait per instruction; hoist
    excess waits onto NoOps inserted before the offender on the same engine."""
    import concourse.mybir as mybir

    ctr = 0
    for fn in nc.m.functions:
        for blk in fn.blocks:
            insts = list(blk.instructions)
            new_insts = []
            changed = False
            for ins in insts:
                si = getattr(ins, "sync_info", None)
                if si is not None and si.on_wait and len(si.on_wait) > max_waits:
                    waits = list(si.on_wait)
                    excess, keep = waits[:-max_waits], waits[-max_waits:]
                    for i in range(0, len(excess), max_waits):
                        ctr += 1
                        nop = mybir.InstNoOp(
                            name=f"waitsplit_{ctr}",
                            engine=ins.engine,
                            sync_info=mybir.SyncInfo(
                                on_wait=excess[i : i + max_waits], on_update=[]
                            ),
                            text_hint="waitsplit",
                        )
                        new_insts.append(nop)
                        nc.register_instruction(nop, overwrite=True)
                    ins.sync_info = mybir.SyncInfo(
                        on_wait=keep, on_update=list(si.on_update)
                    )
                    changed = True
                new_insts.append(ins)
            if changed:
                blk.instructions = new_insts


@functools.cache
def _build():
    import concourse.bass as bass
    import concourse.mybir as mybir
    import concourse.tile as tile

    f32 = mybir.dt.float32
    f32r = mybir.dt.float32r
    bf16 = mybir.dt.bfloat16

    nc = bass.Bass()

    # ---- DRAM parameters (bf16, per-core views prepared on host) ----
    xT = nc.declare_dram_parameter("xT", [C, R], bf16, isOutput=False)
    yT = nc.declare_dram_parameter("yT", [C, RL], bf16, isOutput=False)
    wqT = nc.declare_dram_parameter("wqT", [C, LOCD], bf16, isOutput=False)
    wkT = nc.declare_dram_parameter("wkT", [C, LOCD], bf16, isOutput=False)
    wvT = nc.declare_dram_parameter("wvT", [C, LOCD], bf16, isOutput=False)
    wpT = nc.declare_dram_parameter("wpT", [C, C], bf16, isOutput=False)
    wplT = nc.declare_dram_parameter("wplT", [LOCD, C], bf16, isOutput=False)
    biasb = nc.declare_dram_parameter("biasb", [128, C], f32, isOutput=False)
    onesm = nc.declare_dram_parameter("onesm", [128, 128], bf16, isOutput=False)
    onesf = nc.declare_dram_parameter("onesf", [1, 64], f32r, isOutput=False)
    out_shard = nc.declare_dram_parameter("out_shard", [B, 128, C], f32, isOutput=True)
    out_last = nc.declare_dram_parameter("out_last", [4, 128, C], bf16, isOutput=True)

    # internal DRAM bounce buffers: one full-width AllToAll for batch 0,
    # two half-width (per-u2) AllToAlls for batch 1 so the first hides under
    # u2=1 compute
    a2a_in = [nc.dram_tensor(f"a2a_in{b}", [C, 128], bf16) for b in range(B)]
    a2a_out = [nc.dram_tensor(f"a2a_out{b}", [C, 128], bf16) for b in range(B)]
    a2ah_in = [nc.dram_tensor(f"a2ah_in{u}", [C, 64], bf16) for u in range(2)]
    a2ah_out = [nc.dram_tensor(f"a2ah_out{u}", [C, 64], bf16) for u in range(2)]

    rg = [list(range(NCORES))]

    xTr = xT.rearrange("(kt p) c -> p kt c", p=128)
    yTr = yT.rearrange("(kt p) c -> p kt c", p=128)

    with tile.TileContext(nc) as tc:
        with (
            tc.tile_pool(name="const", bufs=1) as constp,
            tc.tile_pool(name="yx", bufs=3) as ypool,
            tc.tile_pool(name="standing", bufs=1) as stand,
            tc.tile_pool(name="pt", bufs=4) as ptpool,
            tc.tile_pool(name="small", bufs=2) as smallp,
            tc.tile_pool(name="aout", bufs=1) as aoutp,
            tc.tile_pool(name="part", bufs=2) as partp,
            tc.tile_pool(name="psA", bufs=2, space="PSUM") as psA,
            tc.tile_pool(name="psK", bufs=1, space="PSUM") as psK,
            tc.tile_pool(name="psV", bufs=1, space="PSUM") as psV,
        ):
            # ---- constants / weights (gpsimd SWDGE; cheap dispatch).
            # wq first so phase-1 matmuls can start ASAP; wp (2MB, only
            # needed at phase 4) last. ----
            wq_s = constp.tile([128, KT, LOCD], bf16, tag="wq")
            wk_s = constp.tile([128, KT, LOCD], bf16, tag="wk")
            wv_s = constp.tile([128, KT, LOCD], bf16, tag="wv")
            nc.gpsimd.dma_start(wq_s[:], wqT.rearrange("(kt p) m -> p kt m", p=128))
            nc.gpsimd.dma_start(wk_s[:], wkT.rearrange("(kt p) m -> p kt m", p=128))
            nc.gpsimd.dma_start(wv_s[:], wvT.rearrange("(kt p) m -> p kt m", p=128))
            ones_sb = constp.tile([128, 128], bf16)
            nc.gpsimd.dma_start(ones_sb[:], onesm[:])
            ones1 = constp.tile([1, 64], f32r)
            nc.gpsimd.dma_start(ones1[:], onesf[:])
            bias_s = constp.tile([128, C], f32)
            wp_s = constp.tile([128, KT, C], bf16, tag="wp")
            wpl_s = constp.tile([128, C], bf16, tag="wpl")

            # ---- standing tensors ----
            qT_s = stand.tile([128, R], bf16, tag="qT")        # [locdim, (b,n)]
            kT_s = stand.tile([128, RL], bf16, tag="kT")       # [locdim, (b,l)]
            v_s = stand.tile([128, RL // 128, 130], bf16, tag="v")  # [l%128, LT, 2x65]
            ahat_s = stand.tile([128, R], bf16, tag="ahat")    # [locdim, (b,n)]
            # ones columns of v_aug (cols 64 and 129)
            ones_cols = v_s[:, :, 0:130].rearrange("p t (a c) -> p t a c", a=2, c=65)[
                :, :, :, 64:65
            ]
            nc.vector.tensor_copy(
                out=ones_cols,
                in_=ones_sb[:].rearrange("p (t a one) -> p t a one", t=64, a=2, one=1),
            )

            # ---- phase 1: qT projection, 512-col units ----
            def phase1():
                for u in range(R // 512):
                    src = ypool.tile([128, KT, 512], bf16, tag="yx", name=f"xsrc{u}")
                    nc.sync.dma_start(src[:], xTr[:, :, u * 512 : (u + 1) * 512])
                    acc = psA.tile([128, 2, 512], f32, tag="kv", name=f"qacc{u}")
                    for kt in range(KT):
                        nc.tensor.matmul(
                            acc[:, 0, :],
                            lhsT=wq_s[:, kt, :],
                            rhs=src[:, kt, :],
                            start=(kt == 0),
                            stop=(kt == KT - 1),
                        )
                    nc.vector.tensor_copy(
                        out=qT_s[:, u * 512 : (u + 1) * 512], in_=acc[:, 0, :]
                    )

            # ---- phase 2: kT projection + v direct-to-natural, one 512-unit ----
            def phase2_load(b, u):
                off = b * L + u * 512
                src = ypool.tile([128, KT, 512], bf16, tag="yx", name=f"ysrc{b}_{u}")
                nc.sync.dma_start(src[:], yTr[:, :, off : off + 512])
                return src

            def phase2_unit(b, u, pool, src=None):
                off = b * L + u * 512
                if src is None:
                    src = phase2_load(b, u)
                acc = pool.tile([128, 2, 512], f32, tag="kv", name=f"kvacc{b}_{u}")
                for kt in range(KT):
                    nc.tensor.matmul(
                        acc[:, 0, :],
                        lhsT=wk_s[:, kt, :],
                        rhs=src[:, kt, :],
                        start=(kt == 0),
                        stop=(kt == KT - 1),
                    )
                vv = acc[:, 1, :].rearrange("p (j l) -> p j l", j=4, l=128)
                for j in range(4):
                    for kt in range(KT):
                        nc.tensor.matmul(
                            vv[:, j, :],
                            lhsT=src[:, kt, j * 128 : (j + 1) * 128],
                            rhs=wv_s[:, kt, :],
                            start=(kt == 0),
                            stop=(kt == KT - 1),
                        )
                nc.vector.tensor_copy(out=kT_s[:, off : off + 512], in_=acc[:, 0, :])
                LT0 = off // 128
                nc.vector.tensor_copy(
                    out=v_s[:, LT0 : LT0 + 4, 0:130].rearrange(
                        "p t (a c) -> p t a c", a=2, c=65
                    )[:, :, :, 0:64],
                    in_=acc[:, 1, :].rearrange("p (t a c) -> p t a c", t=4, a=2, c=64),
                )

            # ---- phase 3: attention for batch b, scores->exp->AV pipelined.
            # filler() is called between lt steps to emit PE filler work. ----
            def emit_norm(b, u2, av):
                ncol = b * N + u2 * 512
                for h in range(2):
                    recip = smallp.tile(
                        [1, 512], f32r, tag="recip", name=f"rc{b}_{u2}_{h}"
                    )
                    with nc.allow_low_precision(
                        reason="f32r reciprocal feeds f32r broadcast matmul"
                    ):
                        nc.vector.reciprocal(recip[:], av[64:65, h, :])
                    bc = psA.tile([128, 2, 512], f32, tag="kv", name=f"bc{b}_{u2}_{h}")
                    nc.tensor.matmul(
                        bc[0:64, 0, :],
                        lhsT=ones1[:],
                        rhs=recip[:],
                        start=True,
                        stop=True,
                    )
                    bcst = smallp.tile([64, 512], f32, tag="bcst", name=f"bcs{b}_{u2}_{h}")
                    nc.vector.tensor_copy(out=bcst[:], in_=bc[0:64, 0, :])
                    nc.vector.tensor_mul(
                        out=ahat_s[h * 64 : (h + 1) * 64, ncol : ncol + 512],
                        in0=av[0:64, h, :],
                        in1=bcst[:],
                    )

            def phase3(b, filler=None, u2_hook=None, filler_slots=()):
                filler_slots = set(filler_slots)
                for u2 in range(2):
                    ncol = b * N + u2 * 512
                    av = psV.tile([128, 2, 512], f32, tag="av", name=f"av{b}_{u2}")
                    pts = {}
                    for lt in range(32):
                        koff = b * L + lt * 128
                        st = psA.tile(
                            [128, 2, 512], f32, tag="kv", name=f"st{b}_{u2}_{lt}"
                        )
                        pt = ptpool.tile(
                            [128, 2, 512], bf16, tag="pt", name=f"pt{b}_{u2}_{lt}"
                        )
                        pts[lt] = pt
                        for h in range(2):
                            nc.tensor.matmul(
                                st[:, h, :],
                                lhsT=kT_s[h * 64 : (h + 1) * 64, koff : koff + 128],
                                rhs=qT_s[h * 64 : (h + 1) * 64, ncol : ncol + 512],
                                start=True,
                                stop=True,
                            )
                        nc.scalar.activation(
                            pt[:], st[:], mybir.ActivationFunctionType.Exp, scale=SCALE
                        )
                        if lt > 0:
                            emit_av(b, u2, av, pts[lt - 1], lt - 1)
                        if filler is not None and (u2, lt) in filler_slots:
                            filler()
                    emit_av(b, u2, av, pts[31], 31)
                    emit_norm(b, u2, av)
                    if u2_hook is not None:
                        u2_hook(u2)

            def emit_av(b, u2, av, pt, lt):
                for h in range(2):
                    nc.tensor.matmul(
                        av[0:65, h, :],
                        lhsT=v_s[:, b * 32 + lt, h * 65 : h * 65 + 65],
                        rhs=pt[:, h, :],
                        start=(lt == 0),
                        stop=(lt == 31),
                    )

            # ---- phase 4: AllToAll + output projection for batch b ----
            def phase4_comm(b):
                nc.gpsimd.dma_start(
                    a2a_in[b].rearrange("(j p) n -> p j n", p=128),
                    ahat_s[:, b * N : (b + 1) * N].rearrange(
                        "p (j n) -> p j n", j=8, n=128
                    ),
                )
                nc.gpsimd.collective_compute(
                    "AllToAll",
                    mybir.AluOpType.bypass,
                    replica_groups=rg,
                    ins=[a2a_in[b][:].opt()],
                    outs=[a2a_out[b][:].opt()],
                )
                aout = aoutp.tile([128, KT, 128], bf16, tag="aout", name=f"aout{b}")
                nc.gpsimd.dma_start(
                    aout[:], a2a_out[b].rearrange("(kt p) n -> p kt n", p=128)
                )
                return aout

            def phase4_comm_half(u2):
                nc.gpsimd.dma_start(
                    a2ah_in[u2].rearrange("(j p) n -> p j n", p=128),
                    ahat_s[:, N + u2 * 512 : N + (u2 + 1) * 512].rearrange(
                        "p (j n) -> p j n", j=8, n=64
                    ),
                )
                nc.gpsimd.collective_compute(
                    "AllToAll",
                    mybir.AluOpType.bypass,
                    replica_groups=rg,
                    ins=[a2ah_in[u2][:].opt()],
                    outs=[a2ah_out[u2][:].opt()],
                )
                aout = aoutp.tile(
                    [128, KT, 64], bf16, tag=f"aoh{u2}", name=f"aouth{u2}"
                )
                nc.gpsimd.dma_start(
                    aout[:], a2ah_out[u2].rearrange("(kt p) n -> p kt n", p=128)
                )
                return aout

            def phase4_proj_b1u0(aout):
                pp = psK.tile([128, 2, 512], f32, tag="kv", name="ppb1")
                for cb in range(2):
                    for kt in range(KT):
                        nc.tensor.matmul(
                            pp[0:64, cb, :],
                            lhsT=aout[:, kt, :],
                            rhs=wp_s[:, kt, cb * 512 : (cb + 1) * 512],
                            start=(kt == 0),
                            stop=(kt == KT - 1),
                        )
                part = partp.tile([128, C], f32, tag="part", name="partb1")
                nc.vector.tensor_add(
                    out=part[0:64, :].rearrange("p (a c) -> p a c", a=2, c=512),
                    in0=pp[0:64, :, :],
                    in1=bias_s[0:64, :].rearrange("p (a c) -> p a c", a=2, c=512),
                )
                nc.gpsimd.dma_start(out_shard[1, 0:64, :], part[0:64, :])

            def phase4_last():
                # final n-block (batch 1, cols 512:1024): per-core partial
                # projection over local dims; the 8 partials are summed on
                # the host (avoids one fully-exposed collective at the tail)
                for ch in range(4):
                    off = N + 512 + ch * 128
                    pp2 = psA.tile([128, 2, 512], f32, tag="kv", name=f"pl{ch}")
                    for cb in range(2):
                        nc.tensor.matmul(
                            pp2[:, cb, :],
                            lhsT=ahat_s[:, off : off + 128],
                            rhs=wpl_s[:, cb * 512 : (cb + 1) * 512],
                            start=True,
                            stop=True,
                        )
                    part = partp.tile([128, C], bf16, tag="partl", name=f"plp{ch}")
                    nc.scalar.copy(
                        out=part[:].rearrange("p (a c) -> p a c", a=2, c=512),
                        in_=pp2[:],
                    )
                    nc.gpsimd.dma_start(out_last[ch], part[:])

            def phase4_proj(b, aout):
                pp = psK.tile([128, 2, 512], f32, tag="kv", name=f"pp{b}")
                for cb in range(2):
                    for kt in range(KT):
                        nc.tensor.matmul(
                            pp[:, cb, :],
                            lhsT=aout[:, kt, :],
                            rhs=wp_s[:, kt, cb * 512 : (cb + 1) * 512],
                            start=(kt == 0),
                            stop=(kt == KT - 1),
                        )
                part = partp.tile([128, C], f32, tag="part", name=f"part{b}")
                nc.vector.tensor_add(
                    out=part[:].rearrange("p (a c) -> p a c", a=2, c=512),
                    in0=pp[:],
                    in1=bias_s[:].rearrange("p (a c) -> p a c", a=2, c=512),
                )
                nc.gpsimd.dma_start(out_shard[b], part[:])

            phase1()
            for u in range(8):
                phase2_unit(0, u, psA if u % 2 == 0 else psK)
            # bias/wp are first needed at phase 4; holding them back keeps
            # the DMA device free for the x/y streams during phase 1
            with tc.tile_wait_until(0.024):
                nc.gpsimd.dma_start(bias_s[:], biasb[:])
                nc.gpsimd.dma_start(
                    wp_s[:], wpT.rearrange("(kt p) c -> p kt c", p=128)
                )
                nc.gpsimd.dma_start(wpl_s[:], wplT[:])

            # batch-1 kv units fill batch-0/1 attention PE gaps; their y
            # loads are prefetched one filler slot ahead
            p2b1 = iter(range(8))
            pending = []

            def seed():
                u = next(p2b1, None)
                if u is not None:
                    pending.append((u, phase2_load(1, u)))

            def filler():
                if pending:
                    u, src = pending.pop(0)
                    seed()
                    phase2_unit(1, u, psK, src=src)

            seed()

            phase3(0, filler=filler, filler_slots=[(0, 5), (0, 21), (1, 5), (1, 21)])
            aout0 = phase4_comm(0)  # collective overlaps batch-1 compute
            aout_h = {}

            def u2_hook(u2):
                if u2 == 0:
                    aout_h[u2] = phase4_comm_half(u2)
                    # batch-0 projection emitted mid-batch-1 so its psK tile
                    # doesn't gate the batch-1 fillers on the collective
                    phase4_proj(0, aout0)
                else:
                    phase4_last()

            phase3(
                1,
                filler=filler,
                u2_hook=u2_hook,
                filler_slots=[(0, 1), (0, 5), (0, 9), (0, 13)],
            )
            phase4_proj_b1u0(aout_h[0])

    _split_excess_waits(nc)
    return nc


def _prep_inputs(x, y, Wq, Wk, Wv, Wp, bp):
    import ml_dtypes

    bf16 = ml_dtypes.bfloat16
    x = np.asarray(x, np.float32)
    y = np.asarray(y, np.float32)
    xT = np.ascontiguousarray(x.reshape(R, C).T.astype(bf16))
    yT = np.ascontiguousarray(y.reshape(RL, C).T.astype(bf16))
    WpT = np.ascontiguousarray(np.asarray(Wp, np.float32).T.astype(bf16))
    bias = np.ascontiguousarray(
        np.broadcast_to(np.asarray(bp, np.float32), (128, C))
    )
    ones128 = np.ones((128, 128), bf16)
    ones64 = np.ones((1, 64), np.float32)
    in_maps = []
    for i in range(NCORES):
        sl = slice(i * LOCD, (i + 1) * LOCD)
        in_maps.append(
            {
                "xT": xT,
                "yT": yT,
                "wqT": np.ascontiguousarray(np.asarray(Wq, np.float32)[sl, :].T.astype(bf16)),
                "wkT": np.ascontiguousarray(np.asarray(Wk, np.float32)[sl, :].T.astype(bf16)),
                "wvT": np.ascontiguousarray(np.asarray(Wv, np.float32)[sl, :].T.astype(bf16)),
                "wpT": WpT,
                "wplT": np.ascontiguousarray(WpT[sl, :]),
                "biasb": bias,
                "onesm": ones128,
                "onesf": ones64,
            }
        )
    return in_maps


SIM_OUTPUTS = ["out_shard", "out_last"]


def assemble_from_core_outputs(outs, inputs):
    shards = outs["out_shard"]  # list of [B, 128, C] per core
    lasts = outs["out_last"]    # list of [4, 128, C] per core (partials)
    bp = np.asarray(inputs["bp"], np.float32)
    out = np.zeros((B, N, C), np.float32)
    for j in range(NCORES):
        # batch 0: one full-width AllToAll -> contiguous 128-row block
        out[0, j * 128 : (j + 1) * 128, :] = shards[j][0]
        # batch 1 cols 0:512: half-width AllToAll -> 64-row strip per core
        out[1, j * 64 : (j + 1) * 64, :] = shards[j][1][0:64]
    # batch 1 cols 512:1024: sum of per-core partial projections + bias
    acc = np.zeros((512, C), np.float32)
    for j in range(NCORES):
        acc += np.asarray(lasts[j], np.float32).reshape(512, C)
    out[1, 512:1024, :] = acc + bp
    return out


def kernel(x, y, Wq, Wk, Wv, Wp, bp):
    from concourse.bass_utils import run_bass_kernel_spmd

    nc = _build()
    in_maps = _prep_inputs(x, y, Wq, Wk, Wv, Wp, bp)
    res = run_bass_kernel_spmd(nc, in_maps, list(range(NCORES)))
    outs = {
        "out_shard": [res.results[j]["out_shard"] for j in range(NCORES)],
        "out_last": [res.results[j]["out_last"] for j in range(NCORES)],
    }
    return assemble_from_core_outputs(outs, {"bp": bp})


# revision 31
# speedup vs baseline: 1.4621x; 1.4621x over previous
"""Trainium2 Bass kernel for nn_CrossAttention (B=2, N=1024, L=4096, C=1024, H=16).

Single fused program, head-parallel across 8 NeuronCores (2 heads per core),
bf16 matmuls with f32 PSUM accumulation:
  - q/k projections as [dim, row] matmuls; v projected directly into natural
    [row, dim] layout (no PE transposes).
  - full NxL attention for the core's 2 heads; softmax denominator via an
    appended ones-column in V; exp on the scalar engine (bf16 out), with the
    AV matmuls software-pipelined one step behind the score matmuls so the
    scalar engine never starves the PE.
  - batch-1 k/v projection units are interleaved into both batches' attention
    loops (with prefetched y loads) to fill ACT-bound PE gaps.
  - normalized attention outputs for batch 0 (full width) and batch 1 cols
    0:512 (half width) are exchanged with in-kernel AllToAlls that overlap
    later compute; each core projects its row slice with the full Wp.
  - the final block (batch 1 cols 512:1024) skips the tail-exposed collective:
    each core emits its partial projection over its 128 local dims and the
    host sums the 8 partials (+bias) for those 512 rows.
"""

import functools

import numpy as np

B, N, L, C = 2, 1024, 4096, 1024
H, D = 16, 64
SCALE = D ** -0.5
NCORES = 8
LOCD = C // NCORES       # 128 local head-dims per core (2 heads x 64)
R = B * N                # 2048 query rows
RL = B * L               # 8192 key rows
KT = C // 128            # 8 contraction tiles


def _split_excess_waits(nc, max_waits=1):
    """walrus in this container rejects >1 sync wait per instruction; hoist
    excess waits onto NoOps inserted before the offender on the same engine."""
    import concourse.mybir as mybir

    ctr = 0
    for fn in nc.m.functions:
        for blk in fn.blocks:
            insts = list(blk.instructions)
            new_insts = []
            changed = False
            for ins in insts:
                si = getattr(ins, "sync_info", None)
                if si is not None and si.on_wait and len(si.on_wait) > max_waits:
                    waits = list(si.on_wait)
                    excess, keep = waits[:-max_waits], waits[-max_waits:]
                    for i in range(0, len(excess), max_waits):
                        ctr += 1
                        nop = mybir.InstNoOp(
                            name=f"waitsplit_{ctr}",
                            engine=ins.engine,
                            sync_info=mybir.SyncInfo(
                                on_wait=excess[i : i + max_waits], on_update=[]
                            ),
                            text_hint="waitsplit",
                        )
                        new_insts.append(nop)
                        nc.register_instruction(nop, overwrite=True)
                    ins.sync_info = mybir.SyncInfo(
                        on_wait=keep, on_update=list(si.on_update)
                    )
                    changed = True
                new_insts.append(ins)
            if changed:
                blk.instructions = new_insts


@functools.cache
def _build():
    import concourse.bass as bass
    import concourse.mybir as mybir
    import concourse.tile as tile

    f32 = mybir.dt.float32
    f32r = mybir.dt.float32r
    bf16 = mybir.dt.bfloat16

    nc = bass.Bass()

    # ---- DRAM parameters (bf16, per-core views prepared on host) ----
    xT = nc.declare_dram_parameter("xT", [C, R], bf16, isOutput=False)
    yT = nc.declare_dram_parameter("yT", [C, RL], bf16, isOutput=False)
    wqT = nc.declare_dram_parameter("wqT", [C, LOCD], bf16, isOutput=False)
    wkT = nc.declare_dram_parameter("wkT", [C, LOCD], bf16, isOutput=False)
    wvT = nc.declare_dram_parameter("wvT", [C, LOCD], bf16, isOutput=False)
    wpT = nc.declare_dram_parameter("wpT", [C, C], bf16, isOutput=False)
    wplT = nc.declare_dram_parameter("wplT", [LOCD, C], bf16, isOutput=False)
    biasb = nc.declare_dram_parameter("biasb", [128, C], f32, isOutput=False)
    onesm = nc.declare_dram_parameter("onesm", [128, 128], bf16, isOutput=False)
    onesf = nc.declare_dram_parameter("onesf", [1, 64], f32r, isOutput=False)
    out_shard = nc.declare_dram_parameter("out_shard", [B, 128, C], f32, isOutput=True)
    out_last = nc.declare_dram_parameter("out_last", [4, 128, C], bf16, isOutput=True)

    # internal DRAM bounce buffers: one full-width AllToAll for batch 0,
    # two half-width (per-u2) AllToAlls for batch 1 so the first hides under
    # u2=1 compute
    a2a_in = [nc.dram_tensor(f"a2a_in{b}", [C, 128], bf16) for b in range(B)]
    a2a_out = [nc.dram_tensor(f"a2a_out{b}", [C, 128], bf16) for b in range(B)]
    a2ah_in = [nc.dram_tensor(f"a2ah_in{u}", [C, 64], bf16) for u in range(2)]
    a2ah_out = [nc.dram_tensor(f"a2ah_out{u}", [C, 64], bf16) for u in range(2)]

    rg = [list(range(NCORES))]

    xTr = xT.rearrange("(kt p) c -> p kt c", p=128)
    yTr = yT.rearrange("(kt p) c -> p kt c", p=128)

    with tile.TileContext(nc) as tc:
        with (
            tc.tile_pool(name="const", bufs=1) as constp,
            tc.tile_pool(name="yx", bufs=3) as ypool,
            tc.tile_pool(name="standing", bufs=1) as stand,
            tc.tile_pool(name="pt", bufs=4) as ptpool,
            tc.tile_pool(name="small", bufs=2) as smallp,
            tc.tile_pool(name="aout", bufs=1) as aoutp,
            tc.tile_pool(name="part", bufs=2) as partp,
            tc.tile_pool(name="psA", bufs=2, space="PSUM") as psA,
            tc.tile_pool(name="psK", bufs=1, space="PSUM") as psK,
            tc.tile_pool(name="psV", bufs=1, space="PSUM") as psV,
        ):
            # ---- constants / weights (gpsimd SWDGE; cheap dispatch).
            # wq first so phase-1 matmuls can start ASAP; wp (2MB, only
            # needed at phase 4) last. ----
            wq_s = constp.tile([128, KT, LOCD], bf16, tag="wq")
            wk_s = constp.tile([128, KT, LOCD], bf16, tag="wk")
            wv_s = constp.tile([128, KT, LOCD], bf16, tag="wv")
            nc.gpsimd.dma_start(wq_s[:], wqT.rearrange("(kt p) m -> p kt m", p=128))
            nc.gpsimd.dma_start(wk_s[:], wkT.rearrange("(kt p) m -> p kt m", p=128))
            nc.gpsimd.dma_start(wv_s[:], wvT.rearrange("(kt p) m -> p kt m", p=128))
            ones_sb = constp.tile([128, 128], bf16)
            nc.gpsimd.dma_start(ones_sb[:], onesm[:])
            ones1 = constp.tile([1, 64], f32r)
            nc.gpsimd.dma_start(ones1[:], onesf[:])
            bias_s = constp.tile([128, C], f32)
            wp_s = constp.tile([128, KT, C], bf16, tag="wp")
            wpl_s = constp.tile([128, C], bf16, tag="wpl")

            # ---- standing tensors ----
            qT_s = stand.tile([128, R], bf16, tag="qT")        # [locdim, (b,n)]
            kT_s = stand.tile([128, RL], bf16, tag="kT")       # [locdim, (b,l)]
            v_s = stand.tile([128, RL // 128, 130], bf16, tag="v")  # [l%128, LT, 2x65]
            ahat_s = stand.tile([128, R], bf16, tag="ahat")    # [locdim, (b,n)]
            # ones columns of v_aug (cols 64 and 129)
            ones_cols = v_s[:, :, 0:130].rearrange("p t (a c) -> p t a c", a=2, c=65)[
                :, :, :, 64:65
            ]
            nc.vector.tensor_copy(
                out=ones_cols,
                in_=ones_sb[:].rearrange("p (t a one) -> p t a one", t=64, a=2, one=1),
            )

            # ---- phase 1: qT projection, 512-col units ----
            def phase1_unit(u, pool):
                src = ypool.tile([128, KT, 512], bf16, tag="yx", name=f"xsrc{u}")
                nc.sync.dma_start(src[:], xTr[:, :, u * 512 : (u + 1) * 512])
                acc = pool.tile([128, 2, 512], f32, tag="kv", name=f"qacc{u}")
                for kt in range(KT):
                    nc.tensor.matmul(
                        acc[:, 0, :],
                        lhsT=wq_s[:, kt, :],
                        rhs=src[:, kt, :],
                        start=(kt == 0),
                        stop=(kt == KT - 1),
                    )
                nc.vector.tensor_copy(
                    out=qT_s[:, u * 512 : (u + 1) * 512], in_=acc[:, 0, :]
                )

            # ---- phase 2: kT projection + v direct-to-natural, one 512-unit ----
            def phase2_load(b, u):
                off = b * L + u * 512
                src = ypool.tile([128, KT, 512], bf16, tag="yx", name=f"ysrc{b}_{u}")
                nc.sync.dma_start(src[:], yTr[:, :, off : off + 512])
                return src

            def phase2_unit(b, u, pool, src=None):
                off = b * L + u * 512
                if src is None:
                    src = phase2_load(b, u)
                acc = pool.tile([128, 2, 512], f32, tag="kv", name=f"kvacc{b}_{u}")
                for kt in range(KT):
                    nc.tensor.matmul(
                        acc[:, 0, :],
                        lhsT=wk_s[:, kt, :],
                        rhs=src[:, kt, :],
                        start=(kt == 0),
                        stop=(kt == KT - 1),
                    )
                vv = acc[:, 1, :].rearrange("p (j l) -> p j l", j=4, l=128)
                for j in range(4):
                    for kt in range(KT):
                        nc.tensor.matmul(
                            vv[:, j, :],
                            lhsT=src[:, kt, j * 128 : (j + 1) * 128],
                            rhs=wv_s[:, kt, :],
                            start=(kt == 0),
                            stop=(kt == KT - 1),
                        )
                nc.vector.tensor_copy(out=kT_s[:, off : off + 512], in_=acc[:, 0, :])
                LT0 = off // 128
                nc.vector.tensor_copy(
                    out=v_s[:, LT0 : LT0 + 4, 0:130].rearrange(
                        "p t (a c) -> p t a c", a=2, c=65
                    )[:, :, :, 0:64],
                    in_=acc[:, 1, :].rearrange("p (t a c) -> p t a c", t=4, a=2, c=64),
                )

            # ---- phase 3: attention for batch b, scores->exp->AV pipelined.
            # filler() is called between lt steps to emit PE filler work. ----
            def emit_norm(b, u2, av):
                ncol = b * N + u2 * 512
                for h in range(2):
                    recip = smallp.tile(
                        [1, 512], f32r, tag="recip", name=f"rc{b}_{u2}_{h}"
                    )
                    with nc.allow_low_precision(
                        reason="f32r reciprocal feeds f32r broadcast matmul"
                    ):
                        nc.vector.reciprocal(recip[:], av[64:65, h, :])
                    bc = psA.tile([128, 2, 512], f32, tag="kv", name=f"bc{b}_{u2}_{h}")
                    nc.tensor.matmul(
                        bc[0:64, 0, :],
                        lhsT=ones1[:],
                        rhs=recip[:],
                        start=True,
                        stop=True,
                    )
                    bcst = smallp.tile([64, 512], f32, tag="bcst", name=f"bcs{b}_{u2}_{h}")
                    nc.vector.tensor_copy(out=bcst[:], in_=bc[0:64, 0, :])
                    nc.vector.tensor_mul(
                        out=ahat_s[h * 64 : (h + 1) * 64, ncol : ncol + 512],
                        in0=av[0:64, h, :],
                        in1=bcst[:],
                    )

            def phase3(b, filler=None, u2_hook=None, filler_slots=()):
                filler_slots = set(filler_slots)
                for u2 in range(2):
                    ncol = b * N + u2 * 512
                    av = psV.tile([128, 2, 512], f32, tag="av", name=f"av{b}_{u2}")
                    pts = {}
                    for lt in range(32):
                        koff = b * L + lt * 128
                        st = psA.tile(
                            [128, 2, 512], f32, tag="kv", name=f"st{b}_{u2}_{lt}"
                        )
                        pt = ptpool.tile(
                            [128, 2, 512], bf16, tag="pt", name=f"pt{b}_{u2}_{lt}"
                        )
                        pts[lt] = pt
                        for h in range(2):
                            nc.tensor.matmul(
                                st[:, h, :],
                                lhsT=kT_s[h * 64 : (h + 1) * 64, koff : koff + 128],
                                rhs=qT_s[h * 64 : (h + 1) * 64, ncol : ncol + 512],
                                start=True,
                                stop=True,
                            )
                        nc.scalar.activation(
                            pt[:], st[:], mybir.ActivationFunctionType.Exp, scale=SCALE
                        )
                        if lt > 0:
                            emit_av(b, u2, av, pts[lt - 1], lt - 1)
                        if filler is not None and (u2, lt) in filler_slots:
                            filler()
                    emit_av(b, u2, av, pts[31], 31)
                    emit_norm(b, u2, av)
                    if u2_hook is not None:
                        u2_hook(u2)

            def emit_av(b, u2, av, pt, lt):
                for h in range(2):
                    nc.tensor.matmul(
                        av[0:65, h, :],
                        lhsT=v_s[:, b * 32 + lt, h * 65 : h * 65 + 65],
                        rhs=pt[:, h, :],
                        start=(lt == 0),
                        stop=(lt == 31),
                    )

            # ---- phase 4: AllToAll + output projection for batch b ----
            def phase4_comm(b):
                nc.gpsimd.dma_start(
                    a2a_in[b].rearrange("(j p) n -> p j n", p=128),
                    ahat_s[:, b * N : (b + 1) * N].rearrange(
                        "p (j n) -> p j n", j=8, n=128
                    ),
                )
                nc.gpsimd.collective_compute(
                    "AllToAll",
                    mybir.AluOpType.bypass,
                    replica_groups=rg,
                    ins=[a2a_in[b][:].opt()],
                    outs=[a2a_out[b][:].opt()],
                )
                aout = aoutp.tile([128, KT, 128], bf16, tag="aout", name=f"aout{b}")
                nc.gpsimd.dma_start(
                    aout[:], a2a_out[b].rearrange("(kt p) n -> p kt n", p=128)
                )
                return aout

            def phase4_comm_half(u2):
                nc.gpsimd.dma_start(
                    a2ah_in[u2].rearrange("(j p) n -> p j n", p=128),
                    ahat_s[:, N + u2 * 512 : N + (u2 + 1) * 512].rearrange(
                        "p (j n) -> p j n", j=8, n=64
                    ),
                )
                nc.gpsimd.collective_compute(
                    "AllToAll",
                    mybir.AluOpType.bypass,
                    replica_groups=rg,
                    ins=[a2ah_in[u2][:].opt()],
                    outs=[a2ah_out[u2][:].opt()],
                )
                aout = aoutp.tile(
                    [128, KT, 64], bf16, tag=f"aoh{u2}", name=f"aouth{u2}"
                )
                nc.gpsimd.dma_start(
                    aout[:], a2ah_out[u2].rearrange("(kt p) n -> p kt n", p=128)
                )
                return aout

            def phase4_proj_b1u0(aout):
                pp = psK.tile([128, 2, 512], f32, tag="kv", name="ppb1")
                for cb in range(2):
                    for kt in range(KT):
                        nc.tensor.matmul(
                            pp[0:64, cb, :],
                            lhsT=aout[:, kt, :],
                            rhs=wp_s[:, kt, cb * 512 : (cb + 1) * 512],
                            start=(kt == 0),
                            stop=(kt == KT - 1),
                        )
                part = partp.tile([128, C], f32, tag="part", name="partb1")
                nc.vector.tensor_add(
                    out=part[0:64, :].rearrange("p (a c) -> p a c", a=2, c=512),
                    in0=pp[0:64, :, :],
                    in1=bias_s[0:64, :].rearrange("p (a c) -> p a c", a=2, c=512),
                )
                nc.gpsimd.dma_start(out_shard[1, 0:64, :], part[0:64, :])

            def phase4_last():
                # final n-block (batch 1, cols 512:1024): per-core partial
                # projection over local dims; the 8 partials are summed on
                # the host (avoids one fully-exposed collective at the tail)
                for ch in range(4):
                    off = N + 512 + ch * 128
                    pp2 = psA.tile([128, 2, 512], f32, tag="kv", name=f"pl{ch}")
                    for cb in range(2):
                        nc.tensor.matmul(
                            pp2[:, cb, :],
                            lhsT=ahat_s[:, off : off + 128],
                            rhs=wpl_s[:, cb * 512 : (cb + 1) * 512],
                            start=True,
                            stop=True,
                        )
                    part = partp.tile([128, C], bf16, tag="partl", name=f"plp{ch}")
                    nc.scalar.copy(
                        out=part[:].rearrange("p (a c) -> p a c", a=2, c=512),
                        in_=pp2[:],
                    )
                    nc.gpsimd.dma_start(out_last[ch], part[:])

            def phase4_proj(b, aout):
                pp = psK.tile([128, 2, 512], f32, tag="kv", name=f"pp{b}")
                for cb in range(2):
                    for kt in range(KT):
                        nc.tensor.matmul(
                            pp[:, cb, :],
                            lhsT=aout[:, kt, :],
                            rhs=wp_s[:, kt, cb * 512 : (cb + 1) * 512],
                            start=(kt == 0),
                            stop=(kt == KT - 1),
                        )
                part = partp.tile([128, C], f32, tag="part", name=f"part{b}")
                nc.vector.tensor_add(
                    out=part[:].rearrange("p (a c) -> p a c", a=2, c=512),
                    in0=pp[:],
                    in1=bias_s[:].rearrange("p (a c) -> p a c", a=2, c=512),
                )
                nc.gpsimd.dma_start(out_shard[b], part[:])

            phase1_unit(0, psA)
            for u in range(8):
                phase2_unit(0, u, psA if u % 2 == 0 else psK)
            # bias/wp are first needed at phase 4; holding them back keeps
            # the DMA device free for the x/y streams during phase 1
            with tc.tile_wait_until(0.024):
                nc.gpsimd.dma_start(bias_s[:], biasb[:])
                nc.gpsimd.dma_start(
                    wp_s[:], wpT.rearrange("(kt p) c -> p kt c", p=128)
                )
                nc.gpsimd.dma_start(wpl_s[:], wplT[:])

            # batch-1 kv units fill batch-0/1 attention PE gaps; their y
            # loads are prefetched one filler slot ahead
            p2b1 = iter(range(8))
            pending = []

            def seed():
                u = next(p2b1, None)
                if u is not None:
                    pending.append((u, phase2_load(1, u)))

            def filler():
                if pending:
                    u, src = pending.pop(0)
                    seed()
                    phase2_unit(1, u, psK, src=src)

            seed()

            # q-units 1-3 just-in-time: unit 1 before u2=1 reads it, units
            # 2-3 (batch 1 queries) before phase3(1)
            qfill = iter([1, 2, 3])

            def filler_q():
                u = next(qfill, None)
                if u is not None:
                    phase1_unit(u, psK)
                else:
                    filler()

            phase3(
                0,
                filler=filler_q,
                filler_slots=[(0, 3), (0, 13), (0, 23), (1, 3), (1, 13), (1, 23)],
            )
            aout0 = phase4_comm(0)  # collective overlaps batch-1 compute
            aout_h = {}

            def u2_hook(u2):
                if u2 == 0:
                    aout_h[u2] = phase4_comm_half(u2)
                    # batch-0 projection emitted mid-batch-1 so its psK tile
                    # doesn't gate the batch-1 fillers on the collective
                    phase4_proj(0, aout0)
                else:
                    phase4_last()

            phase3(
                1,
                filler=filler,
                u2_hook=u2_hook,
                filler_slots=[(0, 1), (0, 5), (0, 9), (0, 13), (0, 17)],
            )
            phase4_proj_b1u0(aout_h[0])

    _split_excess_waits(nc)
    return nc


def _prep_inputs(x, y, Wq, Wk, Wv, Wp, bp):
    import ml_dtypes

    bf16 = ml_dtypes.bfloat16
    x = np.asarray(x, np.float32)
    y = np.asarray(y, np.float32)
    xT = np.ascontiguousarray(x.reshape(R, C).T.astype(bf16))
    yT = np.ascontiguousarray(y.reshape(RL, C).T.astype(bf16))
    WpT = np.ascontiguousarray(np.asarray(Wp, np.float32).T.astype(bf16))
    bias = np.ascontiguousarray(
        np.broadcast_to(np.asarray(bp, np.float32), (128, C))
    )
    ones128 = np.ones((128, 128), bf16)
    ones64 = np.ones((1, 64), np.float32)
    in_maps = []
    for i in range(NCORES):
        sl = slice(i * LOCD, (i + 1) * LOCD)
        in_maps.append(
            {
                "xT": xT,
                "yT": yT,
                "wqT": np.ascontiguousarray(np.asarray(Wq, np.float32)[sl, :].T.astype(bf16)),
                "wkT": np.ascontiguousarray(np.asarray(Wk, np.float32)[sl, :].T.astype(bf16)),
                "wvT": np.ascontiguousarray(np.asarray(Wv, np.float32)[sl, :].T.astype(bf16)),
                "wpT": WpT,
                "wplT": np.ascontiguousarray(WpT[sl, :]),
                "biasb": bias,
                "onesm": ones128,
                "onesf": ones64,
            }
        )
    return in_maps


SIM_OUTPUTS = ["out_shard", "out_last"]


def assemble_from_core_outputs(outs, inputs):
    shards = outs["out_shard"]  # list of [B, 128, C] per core
    lasts = outs["out_last"]    # list of [4, 128, C] per core (partials)
    bp = np.asarray(inputs["bp"], np.float32)
    out = np.zeros((B, N, C), np.float32)
    for j in range(NCORES):
        # batch 0: one full-width AllToAll -> contiguous 128-row block
        out[0, j * 128 : (j + 1) * 128, :] = shards[j][0]
        # batch 1 cols 0:512: half-width AllToAll -> 64-row strip per core
        out[1, j * 64 : (j + 1) * 64, :] = shards[j][1][0:64]
    # batch 1 cols 512:1024: sum of per-core partial projections + bias
    acc = np.zeros((512, C), np.float32)
    for j in range(NCORES):
        acc += np.asarray(lasts[j], np.float32).reshape(512, C)
    out[1, 512:1024, :] = acc + bp
    return out


def kernel(x, y, Wq, Wk, Wv, Wp, bp):
    from concourse.bass_utils import run_bass_kernel_spmd

    nc = _build()
    in_maps = _prep_inputs(x, y, Wq, Wk, Wv, Wp, bp)
    res = run_bass_kernel_spmd(nc, in_maps, list(range(NCORES)))
    outs = {
        "out_shard": [res.results[j]["out_shard"] for j in range(NCORES)],
        "out_last": [res.results[j]["out_last"] for j in range(NCORES)],
    }
    return assemble_from_core_outputs(outs, {"bp": bp})
